# revision 24
# baseline (speedup 1.0000x reference)
"""Trainium2 Bass kernel for the C2F critic head (C51 Bellman projection +
interval-refinement action decode).

Math: the reference's per-row scatter-add projection is a per-batch-element
linear map: projected[b] = p_rows[b] @ W_b, with the hat-function matrix
W_b[j, t] = relu(1 - |b_j - t|), b_j = clip((r_b + d_b*z_j - VMIN)/dz, 0, 50).
The fixup-laden lower/upper scatter in the reference is exactly this hat
function (verified to ~1e-6 rel against the jax reference).

Device mapping (per core, 512 batch elems):
  - host pre-permutes p to [8 chunks][2 bands][51 atoms][32 slots][162 rows]
    so every DMA is a contiguous ~1MB block per partition-band.
  - 4 batch elems run concurrently on the PE array as [51,51] x [51,162]
    matmuls placed in the four 64x64 quadrants (tile_position bases {0,64}).
  - W is generated on-chip: DVE broadcast-subtract + (abs_max, sub 1),
    ScalarE Relu(-x); b is computed once for all 512 elems.
  - PSUM eviction via DVE/ScalarE copies (DMA cannot read PSUM).
"""

import os
import sys

sys.path.insert(0, "/opt/trn_rl_repo")

import numpy as np

import concourse.bass as bass
import concourse.tile as tile
from concourse import mybir
from concourse.bass_utils import run_bass_kernel_spmd

# ---------------------------------------------------------------------------
# problem constants (from the reference module; fixed for this problem)
B = 4096
LEVELS = 3
BINS = 9
ATOMS = 51
ACTION_DIM = 6
ROWS = LEVELS * ACTION_DIM * BINS  # 162
V_MIN = -10.0
V_MAX = 10.0
DELTA_Z = (V_MAX - V_MIN) / (ATOMS - 1)

N_CORES = 8
B_CORE = B // N_CORES            # 512
CHUNK = 64                       # batch elems per chunk
N_CHUNKS = B_CORE // CHUNK       # 8
GROUPS = CHUNK // 4              # 16 4-elem matmul groups per chunk
SLOTS = CHUNK // 2               # 32 elems per band per chunk

F32 = mybir.dt.float32

# ---------------------------------------------------------------------------
# Toolchain workarounds: this walrus build accepts at most ONE ge-mode sync
# wait per instruction and rejects eq-mode waits entirely.  (a) barriers are
# switched to the sem-only (EventSemaphore, ge-wait) form, (b) the Tile exit
# drain's global-clock waits go onto a chain of single-wait NOPs, (c) a
# post-pass splits any remaining multi-wait instruction into single-wait NOPs
# on the same engine.

_PATCHED = False


def _apply_patches():
    global _PATCHED
    if _PATCHED:
        return
    _PATCHED = True

    def _sem_only_meb(self, engines):
        for inst in self._sem_only_all_engine_barrier_insts("aeb"):
            self.engines[inst.engine].add_instruction(inst)

    def _sem_only_aeb(self, *, sem_only=False):
        _sem_only_meb(self, None)

    bass.Bass.multi_engine_barrier = _sem_only_meb
    bass.Bass.all_engine_barrier = _sem_only_aeb

    try:
        from concourse.tile import ScopedClock
    except ImportError:
        from concourse.tile_sem_assignment import ScopedClock

    def _drain_and_barrier(self, tick_clock, wait_clock):
        nc = self.nc
        carrier = nc.sync.nop()
        wait_clock.add_sem_waits(
            carrier.ins, ScopedClock({None: tick_clock.global_clock})
        )
        # the split pass below breaks the carrier's waits into 1-wait nops
        nc.sync.drain()
        nc.all_engine_barrier()
        assert self.sems is not None
        popped = nc._tile_sem_poison_stack.pop()
        assert popped is self._sem_poison
        nc.clear_and_free_semaphores(list(self.sems.allocated().values()))
        nc.all_engine_barrier()

    tile.TileContext._drain_and_barrier = _drain_and_barrier


def _split_multiwaits(nc):
    """Hoist all-but-one sync wait of every instruction onto fresh NOPs
    placed immediately before it on the same engine."""
    ctr = 0
    for f in nc.m.functions:
        for bb in f.blocks:
            insts = bb.instructions
            out = []
            changed = False
            for ins in insts:
                si = ins.sync_info
                waits = list(si.on_wait) if si is not None and si.on_wait else []
                if len(waits) > 1:
                    changed = True
                    for w in waits[:-1]:
                        ctr += 1
                        nop = mybir.InstNoOp(name=f"wsplit-{ctr}", ins=[], outs=[])
                        nop.engine = ins.engine
                        nop.sync_info = mybir.SyncInfo(on_wait=[w], on_update=[])
                        nc.register_instruction(nop)
                        out.append(nop)
                    si.on_wait = [waits[-1]]
                out.append(ins)
            if changed:
                bb.instructions = out


# ---------------------------------------------------------------------------
# device program


def build_program(evict_dve_mod=4):
    """Build the SPMD Bass program (identical on all 8 cores)."""
    _apply_patches()
    nc = bass.Bass(trn_type="TRN2", name="c2f_critic")

    p_in = nc.dram_tensor(
        "p", [N_CHUNKS, 115, SLOTS, ROWS], F32, kind="ExternalInput"
    )
    rd_in = nc.dram_tensor("rd", [128, 2 * (B_CORE + SLOTS)], F32, kind="ExternalInput")
    # consts layout (free axis): 0 -> z2 (support on both partition bands),
    # 1..51 -> iota 0..50, 52 -> 1.0, 53..76 -> low0 tiled, 77..100 -> high0
    consts_in = nc.dram_tensor("consts", [128, 101], F32, kind="ExternalInput")
    act_in = nc.dram_tensor("act", [128, 4 * ACTION_DIM], F32, kind="ExternalInput")

    out_t = nc.dram_tensor(
        "out", [N_CHUNKS, 115, SLOTS, ROWS], F32, kind="ExternalOutput"
    )
    dec_t = nc.dram_tensor("dec", [128, 4 * ACTION_DIM], F32, kind="ExternalOutput")

    INV_DZ = float(1.0 / np.float64(DELTA_Z))  # 2.5 exactly

    with tile.TileContext(nc) as tc:
        with (
            tc.tile_pool(name="const", bufs=1) as const_pool,
            tc.tile_pool(name="bcomp", bufs=1) as b_pool,
            tc.tile_pool(name="inp", bufs=3) as in_pool,
            tc.tile_pool(name="wgen", bufs=2) as w_pool,
            tc.tile_pool(name="wtmp", bufs=2) as wtmp_pool,
            tc.tile_pool(name="outp", bufs=2) as out_pool,
            tc.tile_pool(name="dec", bufs=1) as dec_pool,
            tc.tile_pool(name="ps", bufs=1, space="PSUM") as psum_pool,
        ):
            AL = mybir.AluOpType

            cst = const_pool.tile([128, 101], F32)
            nc.sync.dma_start(cst[:, :], consts_in[:, :])
            one_b = cst[:, 52:53]

            # 4 persistent 2-bank PSUM tiles, manually rotated.  Row-band-0
            # matmuls write bank 0, row-band-64 matmuls write bank 1 —
            # concurrent PE writes to the same bank AND partition range hang
            # the device.  Zero once so evictions read defined data in the
            # junk partition band (51..63).
            ps_tiles = [
                psum_pool.tile([128, 1024], F32, tag=f"ps{i}", name=f"psb{i}")
                for i in range(4)
            ]
            for t in ps_tiles:
                nc.vector.memset(t[:, :], 0.0)

            # ---- b = clip((r + d*z_j - VMIN) * INV_DZ, 0, 50), both bands.
            # rd is DMA'd once to partition 0 and broadcast on-chip (Pool);
            # band-64 partitions use columns SHIFTED by SLOTS so one W-gen
            # instruction per chunk covers both bands.
            RDW = B_CORE + SLOTS
            rd_bc = b_pool.tile([128, 2 * RDW], F32, tag="rdbc")
            nc.sync.dma_start(rd_bc[0:115, :], rd_in[0:115, :])
            t1 = b_pool.tile([128, B_CORE], F32, tag="bt1")
            ball = b_pool.tile([128, B_CORE], F32, tag="ball")
            for pa, pz, sh in ((0, 64, 0), (64, 64 + ATOMS, SLOTS)):
                rbc = rd_bc[pa:pz, sh : sh + B_CORE]
                dbc = rd_bc[pa:pz, RDW + sh : RDW + sh + B_CORE]
                # t1 = d * z_j
                nc.vector.tensor_scalar(
                    t1[pa:pz, :], dbc, cst[pa:pz, 0:1], None, AL.mult
                )
                # t1 = t1 + r  (same order as reference's r + d*z)
                nc.vector.tensor_tensor(t1[pa:pz, :], t1[pa:pz, :], rbc, AL.add)
                # ball = (t1 - VMIN) * INV_DZ
                nc.vector.tensor_scalar(
                    ball[pa:pz, :], t1[pa:pz, :], V_MIN, INV_DZ, AL.subtract, AL.mult
                )
                # ball = clip(ball, 0, 50)
                nc.vector.tensor_scalar(
                    ball[pa:pz, :], ball[pa:pz, :], 0.0, float(ATOMS - 1),
                    AL.max, AL.min,
                )

            # ---- decoded (interval-refinement encode+decode, elementwise)
            AD4 = 4 * ACTION_DIM
            act = dec_pool.tile([128, AD4], F32, tag="act")
            nc.sync.dma_start(act[:, :], act_in[:, :])
            low = dec_pool.tile([128, AD4], F32, tag="low")
            high = dec_pool.tile([128, AD4], F32, tag="high")
            sr = dec_pool.tile([128, AD4], F32, tag="sr")
            q = dec_pool.tile([128, AD4], F32, tag="q")
            rsr = dec_pool.tile([128, AD4], F32, tag="rsr")
            cmp = dec_pool.tile([128, AD4 * (BINS - 1)], F32, tag="cmp")
            idx = dec_pool.tile([128, AD4], F32, tag="idx")
            nc.vector.tensor_copy(low[:, :], cst[:, 53 : 53 + AD4])
            nc.vector.tensor_copy(high[:, :], cst[:, 77 : 77 + AD4])
            INV_BINS = float(np.float32(1.0) / np.float32(BINS))
            for _lvl in range(LEVELS):
                # sr = (high - low) / BINS  (via exact-constant reciprocal)
                nc.vector.tensor_tensor(sr[:, :], high[:, :], low[:, :], AL.subtract)
                nc.vector.tensor_scalar(sr[:, :], sr[:, :], INV_BINS, None, AL.mult)
                # q = (act - low) * (1/sr)
                nc.vector.tensor_tensor(q[:, :], act[:, :], low[:, :], AL.subtract)
                nc.vector.reciprocal(rsr[:, :], sr[:, :])
                nc.vector.tensor_tensor(q[:, :], q[:, :], rsr[:, :], AL.mult)
                # idx = clip(floor(q), 0, 8) = sum_m [q >= m], m = 1..8 (exact)
                cv = cmp[:, :].rearrange("p (a m) -> p a m", m=BINS - 1)
                q_b = q[:, :].unsqueeze(2).broadcast_to((128, AD4, BINS - 1))
                thr_b = cst[:, 2:10].unsqueeze(1).broadcast_to((128, AD4, BINS - 1))
                nc.vector.tensor_tensor(cv, q_b, thr_b, AL.is_ge)
                nc.vector.tensor_reduce(
                    idx[:, :], cv, mybir.AxisListType.X, AL.add
                )
                # cont = low + sr*idx ; low = max(-1, cont); high = min(1, cont+sr)
                nc.vector.tensor_tensor(q[:, :], sr[:, :], idx[:, :], AL.mult)
                nc.vector.tensor_tensor(q[:, :], low[:, :], q[:, :], AL.add)
                nc.vector.tensor_scalar(low[:, :], q[:, :], -1.0, None, AL.max)
                nc.vector.tensor_tensor(high[:, :], q[:, :], sr[:, :], AL.add)
                nc.vector.tensor_scalar(high[:, :], high[:, :], 1.0, None, AL.min)
            dec = dec_pool.tile([128, AD4], F32, tag="decout")
            nc.vector.tensor_tensor(dec[:, :], high[:, :], low[:, :], AL.add)
            nc.vector.tensor_scalar(dec[:, :], dec[:, :], 0.5, None, AL.mult)
            nc.gpsimd.dma_start(dec_t[:, :], dec[:, :])

            # ---- main loop over chunks
            for c in range(N_CHUNKS):
                tin = in_pool.tile([128, SLOTS * ROWS], F32, tag="tin")
                nc.sync.dma_start(
                    tin[0:115, :], p_in[c].rearrange("a s r -> a (s r)")
                )

                # W generation, both bands in one pass (partitions 0..114;
                # the junk band 51..63 computes garbage that is never read)
                wt = w_pool.tile([128, SLOTS * ATOMS], F32, tag="wt")
                cols = ball[0:115, c * CHUNK : c * CHUNK + SLOTS]
                diff = wtmp_pool.tile([128, SLOTS * ATOMS], F32, tag="diff")
                dv = diff[0:115, :].rearrange("p (e t) -> p e t", t=ATOMS)
                iota_b = cst[0:115, 1 : 1 + ATOMS].unsqueeze(1).broadcast_to(
                    (115, SLOTS, ATOMS)
                )
                b_b = cols.unsqueeze(2).broadcast_to((115, SLOTS, ATOMS))
                nc.vector.tensor_tensor(dv, iota_b, b_b, AL.subtract)
                # y = |diff| = max(-diff, diff)
                nc.vector.scalar_tensor_tensor(
                    diff[0:115, :], diff[0:115, :], -1.0, diff[0:115, :],
                    AL.mult, AL.max,
                )
                # W = relu(1 - y)
                nc.scalar.activation(
                    wt[0:115, :], diff[0:115, :],
                    mybir.ActivationFunctionType.Relu,
                    bias=one_b[0:115, :], scale=-1.0,
                )
                w0 = wt
                w64 = wt

                tout = out_pool.tile([128, GROUPS * 2 * ROWS], F32, tag="tout")
                for k in range(GROUPS):
                    ps = ps_tiles[(c * GROUPS + k) % 4]
                    s0, s1 = 2 * k, 2 * k + 1
                    # four concurrent [51,51]x[51,162] matmuls, one per quadrant;
                    # row-band-0 pair in bank 0, row-band-64 pair in bank 1
                    nc.tensor.matmul(
                        ps[0:ATOMS, 0:ROWS],
                        w0[0:ATOMS, s0 * ATOMS : (s0 + 1) * ATOMS],
                        tin[0:ATOMS, s0 * ROWS : (s0 + 1) * ROWS],
                    )
                    nc.tensor.matmul(
                        ps[64 : 64 + ATOMS, 0:ROWS],
                        w0[0:ATOMS, s1 * ATOMS : (s1 + 1) * ATOMS],
                        tin[0:ATOMS, s1 * ROWS : (s1 + 1) * ROWS],
                    )
                    nc.tensor.matmul(
                        ps[0:ATOMS, 512 : 512 + ROWS],
                        w64[64 : 64 + ATOMS, s0 * ATOMS : (s0 + 1) * ATOMS],
                        tin[64 : 64 + ATOMS, s0 * ROWS : (s0 + 1) * ROWS],
                    )
                    nc.tensor.matmul(
                        ps[64 : 64 + ATOMS, 512 : 512 + ROWS],
                        w64[64 : 64 + ATOMS, s1 * ATOMS : (s1 + 1) * ATOMS],
                        tin[64 : 64 + ATOMS, s1 * ROWS : (s1 + 1) * ROWS],
                    )
                    dst = tout[0:115, k * 2 * ROWS : (k + 1) * 2 * ROWS].rearrange(
                        "p (b f) -> p b f", b=2
                    )
                    src = ps[0:115, :].rearrange("p (b f) -> p b f", b=2)[
                        :, :, 0:ROWS
                    ]
                    if k % evict_dve_mod == 0:
                        nc.vector.tensor_copy(dst, src)
                    else:
                        nc.scalar.copy(dst, src)

                nc.gpsimd.dma_start(
                    out_t[c].rearrange("a s r -> a (s r)"), tout[0:115, :]
                )

    _split_multiwaits(nc)
    return nc


# ---------------------------------------------------------------------------
# host-side data marshalling

def _build_consts(support, initial_low, initial_high):
    consts = np.zeros((128, 101), dtype=np.float32)
    z = np.asarray(support, dtype=np.float32).reshape(ATOMS)
    consts[0:ATOMS, 0] = z
    consts[64 : 64 + ATOMS, 0] = z
    iota = np.arange(ATOMS, dtype=np.float32)
    consts[:, 1 : 1 + ATOMS] = iota[None, :]
    consts[:, 52] = 1.0
    lo = np.tile(np.asarray(initial_low, np.float32).reshape(ACTION_DIM), 4)
    hi = np.tile(np.asarray(initial_high, np.float32).reshape(ACTION_DIM), 4)
    consts[:, 53:77] = lo[None, :]
    consts[:, 77:101] = hi[None, :]
    return consts


def _prep_core_inputs(p_core, r_core, d_core, a_core, consts):
    """p_core [512,162,51]; r/d [512]; a_core [512,6] -> in_map dict."""
    # dev elem order within a chunk: band0 = 4k+s, band64 = 4k+2+s (k<16,s<2);
    # partition rows 51..63 are padding (never read by the matmuls)
    x = p_core.reshape(N_CHUNKS, GROUPS, 2, 2, ROWS, ATOMS).transpose(0, 2, 5, 1, 3, 4)
    p_dev = np.empty((N_CHUNKS, 115, SLOTS, ROWS), np.float32)
    p_dev[:, 0:ATOMS] = x[:, 0].reshape(N_CHUNKS, ATOMS, SLOTS, ROWS)
    p_dev[:, 64:115] = x[:, 1].reshape(N_CHUNKS, ATOMS, SLOTS, ROWS)
    RDW = B_CORE + SLOTS
    rd1 = np.zeros((2, RDW), np.float32)
    r4 = r_core.reshape(N_CHUNKS, GROUPS, 2, 2).transpose(0, 2, 1, 3)
    d4 = d_core.reshape(N_CHUNKS, GROUPS, 2, 2).transpose(0, 2, 1, 3)
    rd1[0, 0:B_CORE] = r4.reshape(B_CORE)
    rd1[1, 0:B_CORE] = d4.reshape(B_CORE)
    rd1[1, B_CORE:] = 0.5  # pad: harmless nonzero discount
    rd = np.ascontiguousarray(
        np.broadcast_to(rd1.reshape(1, 2 * RDW), (128, 2 * RDW))
    )
    act = np.ascontiguousarray(a_core.reshape(128, 4 * ACTION_DIM))
    return {"p": p_dev, "rd": rd, "consts": consts, "act": act}


def _unpack_core_out(out_dev, dec_dev):
    """out_dev [8,2,51,32,162] -> [512,162,51]; dec [128,24] -> [512,6].
    outband0 slot (k,s2) = elem 4k+2*s2; outband64 = 4k+1+2*s2."""
    y = np.stack([out_dev[:, 0:ATOMS], out_dev[:, 64:115]], axis=1).reshape(
        N_CHUNKS, 2, ATOMS, GROUPS, 2, ROWS
    )
    proj = np.ascontiguousarray(y.transpose(0, 3, 4, 1, 5, 2)).reshape(
        B_CORE, ROWS, ATOMS
    )
    dec = dec_dev.reshape(B_CORE, ACTION_DIM)
    return proj, dec


_CACHED_NC = None


def _get_nc():
    global _CACHED_NC
    if _CACHED_NC is None:
        _CACHED_NC = build_program()
    return _CACHED_NC


def run(inputs, trace=False):
    """Returns ((projected, decoded), exec_time_ns_or_None)."""
    nqp = np.asarray(inputs["next_q_probs"], np.float32)
    reward = np.asarray(inputs["reward"], np.float32).reshape(B)
    discount = np.asarray(inputs["discount"], np.float32).reshape(B)
    caction = np.asarray(inputs["continuous_action"], np.float32)
    support = np.asarray(inputs["support"], np.float32)
    ilow = np.asarray(inputs["initial_low"], np.float32)
    ihigh = np.asarray(inputs["initial_high"], np.float32)

    consts = _build_consts(support, ilow, ihigh)
    p_all = nqp.reshape(N_CORES, B_CORE, ROWS, ATOMS)
    r_all = reward.reshape(N_CORES, B_CORE)
    d_all = discount.reshape(N_CORES, B_CORE)
    a_all = caction.reshape(N_CORES, B_CORE, ACTION_DIM)

    in_maps = [
        _prep_core_inputs(p_all[c], r_all[c], d_all[c], a_all[c], consts)
        for c in range(N_CORES)
    ]

    nc = _get_nc()
    res = run_bass_kernel_spmd(
        nc, in_maps, core_ids=list(range(N_CORES)), trace=trace
    )

    proj = np.empty((B, ROWS, ATOMS), np.float32)
    dec = np.empty((B, ACTION_DIM), np.float32)
    for c in range(N_CORES):
        pc, dc = _unpack_core_out(res.results[c]["out"], res.results[c]["dec"])
        proj[c * B_CORE : (c + 1) * B_CORE] = pc
        dec[c * B_CORE : (c + 1) * B_CORE] = dc

    projected = proj.reshape(B, LEVELS, ACTION_DIM, BINS, ATOMS)
    return (projected, dec), res.exec_time_ns


def kernel(**inputs):
    (projected, decoded), _ = run(inputs, trace=bool(os.environ.get("BASS_KERNEL_TRACE")))
    return projected, decoded


# revision 25
# speedup vs baseline: 1.0078x; 1.0078x over previous
"""Trainium2 Bass kernel for the C2F critic head (C51 Bellman projection +
interval-refinement action decode).

Math: the reference's per-row scatter-add projection is a per-batch-element
linear map: projected[b] = p_rows[b] @ W_b, with the hat-function matrix
W_b[j, t] = relu(1 - |b_j - t|), b_j = clip((r_b + d_b*z_j - VMIN)/dz, 0, 50).
The fixup-laden lower/upper scatter in the reference is exactly this hat
function (verified to ~1e-6 rel against the jax reference).

Device mapping (per core, 512 batch elems, 16 chunks of 32):
  - host pre-permutes p to [chunk][115 partition rows][16 slots][162 rows]
    (atom bands at partition rows 0..50 and 64..114; rows 51..63 padding)
    so each chunk is ONE contiguous ~1.2MB DMA touching 115 partitions —
    DMA bandwidth scales with partitions touched per transfer.
  - 4 batch elems run concurrently on the PE array as [51,51] x [51,162]
    matmuls placed in the four 64x64 quadrants (tile_position bases {0,64}).
  - W is generated on-chip: DVE broadcast-subtract + (abs_max, sub 1),
    ScalarE Relu(-x); b is computed once for all 512 elems.
  - PSUM eviction via DVE/ScalarE copies (DMA cannot read PSUM).
"""

import os
import sys

sys.path.insert(0, "/opt/trn_rl_repo")

import numpy as np

import concourse.bass as bass
import concourse.tile as tile
from concourse import mybir
from concourse.bass_utils import run_bass_kernel_spmd

# ---------------------------------------------------------------------------
# problem constants (from the reference module; fixed for this problem)
B = 4096
LEVELS = 3
BINS = 9
ATOMS = 51
ACTION_DIM = 6
ROWS = LEVELS * ACTION_DIM * BINS  # 162
V_MIN = -10.0
V_MAX = 10.0
DELTA_Z = (V_MAX - V_MIN) / (ATOMS - 1)

N_CORES = 8
B_CORE = B // N_CORES            # 512
CHUNK = 32                       # batch elems per chunk
N_CHUNKS = B_CORE // CHUNK       # 8
GROUPS = CHUNK // 4              # 16 4-elem matmul groups per chunk
SLOTS = CHUNK // 2               # 32 elems per band per chunk

F32 = mybir.dt.float32

# ---------------------------------------------------------------------------
# Toolchain workarounds: this walrus build accepts at most ONE ge-mode sync
# wait per instruction and rejects eq-mode waits entirely.  (a) barriers are
# switched to the sem-only (EventSemaphore, ge-wait) form, (b) the Tile exit
# drain's global-clock waits go onto a chain of single-wait NOPs, (c) a
# post-pass splits any remaining multi-wait instruction into single-wait NOPs
# on the same engine.

_PATCHED = False


def _apply_patches():
    global _PATCHED
    if _PATCHED:
        return
    _PATCHED = True

    def _sem_only_meb(self, engines):
        for inst in self._sem_only_all_engine_barrier_insts("aeb"):
            self.engines[inst.engine].add_instruction(inst)

    def _sem_only_aeb(self, *, sem_only=False):
        _sem_only_meb(self, None)

    bass.Bass.multi_engine_barrier = _sem_only_meb
    bass.Bass.all_engine_barrier = _sem_only_aeb

    try:
        from concourse.tile import ScopedClock
    except ImportError:
        from concourse.tile_sem_assignment import ScopedClock

    def _drain_and_barrier(self, tick_clock, wait_clock):
        nc = self.nc
        carrier = nc.sync.nop()
        wait_clock.add_sem_waits(
            carrier.ins, ScopedClock({None: tick_clock.global_clock})
        )
        # the split pass below breaks the carrier's waits into 1-wait nops
        nc.sync.drain()
        nc.all_engine_barrier()
        assert self.sems is not None
        popped = nc._tile_sem_poison_stack.pop()
        assert popped is self._sem_poison
        nc.clear_and_free_semaphores(list(self.sems.allocated().values()))
        nc.all_engine_barrier()

    tile.TileContext._drain_and_barrier = _drain_and_barrier


def _split_multiwaits(nc):
    """Hoist all-but-one sync wait of every instruction onto fresh NOPs
    placed immediately before it on the same engine."""
    ctr = 0
    for f in nc.m.functions:
        for bb in f.blocks:
            insts = bb.instructions
            out = []
            changed = False
            for ins in insts:
                si = ins.sync_info
                waits = list(si.on_wait) if si is not None and si.on_wait else []
                if len(waits) > 1:
                    changed = True
                    for w in waits[:-1]:
                        ctr += 1
                        nop = mybir.InstNoOp(name=f"wsplit-{ctr}", ins=[], outs=[])
                        nop.engine = ins.engine
                        nop.sync_info = mybir.SyncInfo(on_wait=[w], on_update=[])
                        nc.register_instruction(nop)
                        out.append(nop)
                    si.on_wait = [waits[-1]]
                out.append(ins)
            if changed:
                bb.instructions = out


# ---------------------------------------------------------------------------
# device program


def build_program(evict_dve_mod=4):
    """Build the SPMD Bass program (identical on all 8 cores)."""
    _apply_patches()
    nc = bass.Bass(trn_type="TRN2", name="c2f_critic")

    p_in = nc.dram_tensor(
        "p", [N_CHUNKS, 115, SLOTS, ROWS], F32, kind="ExternalInput"
    )
    rd_in = nc.dram_tensor("rd", [128, 2 * (B_CORE + SLOTS)], F32, kind="ExternalInput")
    # consts layout (free axis): 0 -> z2 (support on both partition bands),
    # 1..51 -> iota 0..50, 52 -> 1.0, 53..76 -> low0 tiled, 77..100 -> high0
    consts_in = nc.dram_tensor("consts", [128, 101], F32, kind="ExternalInput")
    act_in = nc.dram_tensor("act", [128, 4 * ACTION_DIM], F32, kind="ExternalInput")

    out_t = nc.dram_tensor(
        "out", [N_CHUNKS, 115, SLOTS, ROWS], F32, kind="ExternalOutput"
    )
    dec_t = nc.dram_tensor("dec", [128, 4 * ACTION_DIM], F32, kind="ExternalOutput")

    INV_DZ = float(1.0 / np.float64(DELTA_Z))  # 2.5 exactly

    with tile.TileContext(nc) as tc:
        with (
            tc.tile_pool(name="const", bufs=1) as const_pool,
            tc.tile_pool(name="bcomp", bufs=1) as b_pool,
            tc.tile_pool(name="inp", bufs=6) as in_pool,
            tc.tile_pool(name="wgen", bufs=2) as w_pool,
            tc.tile_pool(name="wtmp", bufs=2) as wtmp_pool,
            tc.tile_pool(name="outp", bufs=4) as out_pool,
            tc.tile_pool(name="dec", bufs=1) as dec_pool,
            tc.tile_pool(name="ps", bufs=1, space="PSUM") as psum_pool,
        ):
            AL = mybir.AluOpType

            cst = const_pool.tile([128, 101], F32)
            nc.sync.dma_start(cst[:, :], consts_in[:, :])
            one_b = cst[:, 52:53]

            # 4 persistent 2-bank PSUM tiles, manually rotated.  Row-band-0
            # matmuls write bank 0, row-band-64 matmuls write bank 1 —
            # concurrent PE writes to the same bank AND partition range hang
            # the device.  Zero once so evictions read defined data in the
            # junk partition band (51..63).
            ps_tiles = [
                psum_pool.tile([128, 1024], F32, tag=f"ps{i}", name=f"psb{i}")
                for i in range(4)
            ]
            for t in ps_tiles:
                nc.vector.memset(t[:, :], 0.0)

            # ---- b = clip((r + d*z_j - VMIN) * INV_DZ, 0, 50), both bands.
            # rd is DMA'd once to partition 0 and broadcast on-chip (Pool);
            # band-64 partitions use columns SHIFTED by SLOTS so one W-gen
            # instruction per chunk covers both bands.
            RDW = B_CORE + SLOTS
            rd_bc = b_pool.tile([128, 2 * RDW], F32, tag="rdbc")
            nc.sync.dma_start(rd_bc[0:115, :], rd_in[0:115, :])
            t1 = b_pool.tile([128, B_CORE], F32, tag="bt1")
            ball = b_pool.tile([128, B_CORE], F32, tag="ball")
            for pa, pz, sh in ((0, 64, 0), (64, 64 + ATOMS, SLOTS)):
                rbc = rd_bc[pa:pz, sh : sh + B_CORE]
                dbc = rd_bc[pa:pz, RDW + sh : RDW + sh + B_CORE]
                # t1 = d * z_j
                nc.vector.tensor_scalar(
                    t1[pa:pz, :], dbc, cst[pa:pz, 0:1], None, AL.mult
                )
                # t1 = t1 + r  (same order as reference's r + d*z)
                nc.vector.tensor_tensor(t1[pa:pz, :], t1[pa:pz, :], rbc, AL.add)
                # ball = (t1 - VMIN) * INV_DZ
                nc.vector.tensor_scalar(
                    ball[pa:pz, :], t1[pa:pz, :], V_MIN, INV_DZ, AL.subtract, AL.mult
                )
                # ball = clip(ball, 0, 50)
                nc.vector.tensor_scalar(
                    ball[pa:pz, :], ball[pa:pz, :], 0.0, float(ATOMS - 1),
                    AL.max, AL.min,
                )

            # ---- decoded (interval-refinement encode+decode, elementwise)
            AD4 = 4 * ACTION_DIM
            act = dec_pool.tile([128, AD4], F32, tag="act")
            nc.sync.dma_start(act[:, :], act_in[:, :])
            low = dec_pool.tile([128, AD4], F32, tag="low")
            high = dec_pool.tile([128, AD4], F32, tag="high")
            sr = dec_pool.tile([128, AD4], F32, tag="sr")
            q = dec_pool.tile([128, AD4], F32, tag="q")
            rsr = dec_pool.tile([128, AD4], F32, tag="rsr")
            cmp = dec_pool.tile([128, AD4 * (BINS - 1)], F32, tag="cmp")
            idx = dec_pool.tile([128, AD4], F32, tag="idx")
            nc.vector.tensor_copy(low[:, :], cst[:, 53 : 53 + AD4])
            nc.vector.tensor_copy(high[:, :], cst[:, 77 : 77 + AD4])
            INV_BINS = float(np.float32(1.0) / np.float32(BINS))
            for _lvl in range(LEVELS):
                # sr = (high - low) / BINS  (via exact-constant reciprocal)
                nc.vector.tensor_tensor(sr[:, :], high[:, :], low[:, :], AL.subtract)
                nc.vector.tensor_scalar(sr[:, :], sr[:, :], INV_BINS, None, AL.mult)
                # q = (act - low) * (1/sr)
                nc.vector.tensor_tensor(q[:, :], act[:, :], low[:, :], AL.subtract)
                nc.vector.reciprocal(rsr[:, :], sr[:, :])
                nc.vector.tensor_tensor(q[:, :], q[:, :], rsr[:, :], AL.mult)
                # idx = clip(floor(q), 0, 8) = sum_m [q >= m], m = 1..8 (exact)
                cv = cmp[:, :].rearrange("p (a m) -> p a m", m=BINS - 1)
                q_b = q[:, :].unsqueeze(2).broadcast_to((128, AD4, BINS - 1))
                thr_b = cst[:, 2:10].unsqueeze(1).broadcast_to((128, AD4, BINS - 1))
                nc.vector.tensor_tensor(cv, q_b, thr_b, AL.is_ge)
                nc.vector.tensor_reduce(
                    idx[:, :], cv, mybir.AxisListType.X, AL.add
                )
                # cont = low + sr*idx ; low = max(-1, cont); high = min(1, cont+sr)
                nc.vector.tensor_tensor(q[:, :], sr[:, :], idx[:, :], AL.mult)
                nc.vector.tensor_tensor(q[:, :], low[:, :], q[:, :], AL.add)
                nc.vector.tensor_scalar(low[:, :], q[:, :], -1.0, None, AL.max)
                nc.vector.tensor_tensor(high[:, :], q[:, :], sr[:, :], AL.add)
                nc.vector.tensor_scalar(high[:, :], high[:, :], 1.0, None, AL.min)
            dec = dec_pool.tile([128, AD4], F32, tag="decout")
            nc.vector.tensor_tensor(dec[:, :], high[:, :], low[:, :], AL.add)
            nc.vector.tensor_scalar(dec[:, :], dec[:, :], 0.5, None, AL.mult)
            nc.gpsimd.dma_start(dec_t[:, :], dec[:, :])

            # ---- main loop over chunks
            for c in range(N_CHUNKS):
                tin = in_pool.tile([128, SLOTS * ROWS], F32, tag="tin")
                nc.sync.dma_start(
                    tin[0:115, :], p_in[c].rearrange("a s r -> a (s r)")
                )

                # W generation, both bands in one pass (partitions 0..114;
                # the junk band 51..63 computes garbage that is never read)
                wt = w_pool.tile([128, SLOTS * ATOMS], F32, tag="wt")
                cols = ball[0:115, c * CHUNK : c * CHUNK + SLOTS]
                diff = wtmp_pool.tile([128, SLOTS * ATOMS], F32, tag="diff")
                dv = diff[0:115, :].rearrange("p (e t) -> p e t", t=ATOMS)
                iota_b = cst[0:115, 1 : 1 + ATOMS].unsqueeze(1).broadcast_to(
                    (115, SLOTS, ATOMS)
                )
                b_b = cols.unsqueeze(2).broadcast_to((115, SLOTS, ATOMS))
                nc.vector.tensor_tensor(dv, iota_b, b_b, AL.subtract)
                # y = |diff| = max(-diff, diff)
                nc.vector.scalar_tensor_tensor(
                    diff[0:115, :], diff[0:115, :], -1.0, diff[0:115, :],
                    AL.mult, AL.max,
                )
                # W = relu(1 - y)
                nc.scalar.activation(
                    wt[0:115, :], diff[0:115, :],
                    mybir.ActivationFunctionType.Relu,
                    bias=one_b[0:115, :], scale=-1.0,
                )
                w0 = wt
                w64 = wt

                tout = out_pool.tile([128, GROUPS * 2 * ROWS], F32, tag="tout")
                for k in range(GROUPS):
                    ps = ps_tiles[(c * GROUPS + k) % 4]
                    s0, s1 = 2 * k, 2 * k + 1
                    # four concurrent [51,51]x[51,162] matmuls, one per quadrant;
                    # row-band-0 pair in bank 0, row-band-64 pair in bank 1
                    nc.tensor.matmul(
                        ps[0:ATOMS, 0:ROWS],
                        w0[0:ATOMS, s0 * ATOMS : (s0 + 1) * ATOMS],
                        tin[0:ATOMS, s0 * ROWS : (s0 + 1) * ROWS],
                    )
                    nc.tensor.matmul(
                        ps[64 : 64 + ATOMS, 0:ROWS],
                        w0[0:ATOMS, s1 * ATOMS : (s1 + 1) * ATOMS],
                        tin[0:ATOMS, s1 * ROWS : (s1 + 1) * ROWS],
                    )
                    nc.tensor.matmul(
                        ps[0:ATOMS, 512 : 512 + ROWS],
                        w64[64 : 64 + ATOMS, s0 * ATOMS : (s0 + 1) * ATOMS],
                        tin[64 : 64 + ATOMS, s0 * ROWS : (s0 + 1) * ROWS],
                    )
                    nc.tensor.matmul(
                        ps[64 : 64 + ATOMS, 512 : 512 + ROWS],
                        w64[64 : 64 + ATOMS, s1 * ATOMS : (s1 + 1) * ATOMS],
                        tin[64 : 64 + ATOMS, s1 * ROWS : (s1 + 1) * ROWS],
                    )
                    dst = tout[0:115, k * 2 * ROWS : (k + 1) * 2 * ROWS].rearrange(
                        "p (b f) -> p b f", b=2
                    )
                    src = ps[0:115, :].rearrange("p (b f) -> p b f", b=2)[
                        :, :, 0:ROWS
                    ]
                    if k % evict_dve_mod == 0:
                        nc.vector.tensor_copy(dst, src)
                    else:
                        nc.scalar.copy(dst, src)

                nc.gpsimd.dma_start(
                    out_t[c].rearrange("a s r -> a (s r)"), tout[0:115, :]
                )

    _split_multiwaits(nc)
    return nc


# ---------------------------------------------------------------------------
# host-side data marshalling

def _build_consts(support, initial_low, initial_high):
    consts = np.zeros((128, 101), dtype=np.float32)
    z = np.asarray(support, dtype=np.float32).reshape(ATOMS)
    consts[0:ATOMS, 0] = z
    consts[64 : 64 + ATOMS, 0] = z
    iota = np.arange(ATOMS, dtype=np.float32)
    consts[:, 1 : 1 + ATOMS] = iota[None, :]
    consts[:, 52] = 1.0
    lo = np.tile(np.asarray(initial_low, np.float32).reshape(ACTION_DIM), 4)
    hi = np.tile(np.asarray(initial_high, np.float32).reshape(ACTION_DIM), 4)
    consts[:, 53:77] = lo[None, :]
    consts[:, 77:101] = hi[None, :]
    return consts


def _prep_core_inputs(p_core, r_core, d_core, a_core, consts):
    """p_core [512,162,51]; r/d [512]; a_core [512,6] -> in_map dict."""
    # dev elem order within a chunk: band0 = 4k+s, band64 = 4k+2+s (k<16,s<2);
    # partition rows 51..63 are padding (never read by the matmuls)
    x = p_core.reshape(N_CHUNKS, GROUPS, 2, 2, ROWS, ATOMS).transpose(0, 2, 5, 1, 3, 4)
    p_dev = np.empty((N_CHUNKS, 115, SLOTS, ROWS), np.float32)
    p_dev[:, 0:ATOMS] = x[:, 0].reshape(N_CHUNKS, ATOMS, SLOTS, ROWS)
    p_dev[:, 64:115] = x[:, 1].reshape(N_CHUNKS, ATOMS, SLOTS, ROWS)
    RDW = B_CORE + SLOTS
    rd1 = np.zeros((2, RDW), np.float32)
    r4 = r_core.reshape(N_CHUNKS, GROUPS, 2, 2).transpose(0, 2, 1, 3)
    d4 = d_core.reshape(N_CHUNKS, GROUPS, 2, 2).transpose(0, 2, 1, 3)
    rd1[0, 0:B_CORE] = r4.reshape(B_CORE)
    rd1[1, 0:B_CORE] = d4.reshape(B_CORE)
    rd1[1, B_CORE:] = 0.5  # pad: harmless nonzero discount
    rd = np.ascontiguousarray(
        np.broadcast_to(rd1.reshape(1, 2 * RDW), (128, 2 * RDW))
    )
    act = np.ascontiguousarray(a_core.reshape(128, 4 * ACTION_DIM))
    return {"p": p_dev, "rd": rd, "consts": consts, "act": act}


def _unpack_core_out(out_dev, dec_dev):
    """out_dev [8,2,51,32,162] -> [512,162,51]; dec [128,24] -> [512,6].
    outband0 slot (k,s2) = elem 4k+2*s2; outband64 = 4k+1+2*s2."""
    y = np.stack([out_dev[:, 0:ATOMS], out_dev[:, 64:115]], axis=1).reshape(
        N_CHUNKS, 2, ATOMS, GROUPS, 2, ROWS
    )
    proj = np.ascontiguousarray(y.transpose(0, 3, 4, 1, 5, 2)).reshape(
        B_CORE, ROWS, ATOMS
    )
    dec = dec_dev.reshape(B_CORE, ACTION_DIM)
    return proj, dec


_CACHED_NC = None


def _get_nc():
    global _CACHED_NC
    if _CACHED_NC is None:
        _CACHED_NC = build_program()
    return _CACHED_NC


def run(inputs, trace=False):
    """Returns ((projected, decoded), exec_time_ns_or_None)."""
    nqp = np.asarray(inputs["next_q_probs"], np.float32)
    reward = np.asarray(inputs["reward"], np.float32).reshape(B)
    discount = np.asarray(inputs["discount"], np.float32).reshape(B)
    caction = np.asarray(inputs["continuous_action"], np.float32)
    support = np.asarray(inputs["support"], np.float32)
    ilow = np.asarray(inputs["initial_low"], np.float32)
    ihigh = np.asarray(inputs["initial_high"], np.float32)

    consts = _build_consts(support, ilow, ihigh)
    p_all = nqp.reshape(N_CORES, B_CORE, ROWS, ATOMS)
    r_all = reward.reshape(N_CORES, B_CORE)
    d_all = discount.reshape(N_CORES, B_CORE)
    a_all = caction.reshape(N_CORES, B_CORE, ACTION_DIM)

    in_maps = [
        _prep_core_inputs(p_all[c], r_all[c], d_all[c], a_all[c], consts)
        for c in range(N_CORES)
    ]

    nc = _get_nc()
    res = run_bass_kernel_spmd(
        nc, in_maps, core_ids=list(range(N_CORES)), trace=trace
    )

    proj = np.empty((B, ROWS, ATOMS), np.float32)
    dec = np.empty((B, ACTION_DIM), np.float32)
    for c in range(N_CORES):
        pc, dc = _unpack_core_out(res.results[c]["out"], res.results[c]["dec"])
        proj[c * B_CORE : (c + 1) * B_CORE] = pc
        dec[c * B_CORE : (c + 1) * B_CORE] = dc

    projected = proj.reshape(B, LEVELS, ACTION_DIM, BINS, ATOMS)
    return (projected, dec), res.exec_time_ns


def kernel(**inputs):
    (projected, decoded), _ = run(inputs, trace=bool(os.environ.get("BASS_KERNEL_TRACE")))
    return projected, decoded


# revision 26
# speedup vs baseline: 1.0561x; 1.0480x over previous
"""Trainium2 Bass kernel for the C2F critic head (C51 Bellman projection +
interval-refinement action decode).

Math: the reference's per-row scatter-add projection is a per-batch-element
linear map: projected[b] = p_rows[b] @ W_b, with the hat-function matrix
W_b[j, t] = relu(1 - |b_j - t|), b_j = clip((r_b + d_b*z_j - VMIN)/dz, 0, 50).
The fixup-laden lower/upper scatter in the reference is exactly this hat
function (verified to ~1e-6 rel against the jax reference).

Device mapping (per core, 512 batch elems, 16 chunks of 32):
  - host pre-permutes p to [chunk][115 partition rows][16 slots][162 rows]
    (atom bands at partition rows 0..50 and 64..114; rows 51..63 padding)
    so each chunk is ONE contiguous ~1.2MB DMA touching 115 partitions —
    DMA bandwidth scales with partitions touched per transfer.
  - 4 batch elems run concurrently on the PE array as [51,51] x [51,162]
    matmuls placed in the four 64x64 quadrants (tile_position bases {0,64}).
  - W is generated on-chip: DVE broadcast-subtract + (abs_max, sub 1),
    ScalarE Relu(-x); b is computed once for all 512 elems.
  - PSUM eviction via DVE/ScalarE copies (DMA cannot read PSUM).
"""

import os
import sys

sys.path.insert(0, "/opt/trn_rl_repo")

import numpy as np

import concourse.bass as bass
import concourse.tile as tile
from concourse import mybir
from concourse.bass_utils import run_bass_kernel_spmd

# ---------------------------------------------------------------------------
# problem constants (from the reference module; fixed for this problem)
B = 4096
LEVELS = 3
BINS = 9
ATOMS = 51
ACTION_DIM = 6
ROWS = LEVELS * ACTION_DIM * BINS  # 162
V_MIN = -10.0
V_MAX = 10.0
DELTA_Z = (V_MAX - V_MIN) / (ATOMS - 1)

N_CORES = 8
B_CORE = B // N_CORES            # 512
CHUNK = 32                       # batch elems per chunk
N_CHUNKS = B_CORE // CHUNK       # 8
GROUPS = CHUNK // 4              # 16 4-elem matmul groups per chunk
SLOTS = CHUNK // 2               # 32 elems per band per chunk

F32 = mybir.dt.float32

# ---------------------------------------------------------------------------
# Toolchain workarounds: this walrus build accepts at most ONE ge-mode sync
# wait per instruction and rejects eq-mode waits entirely.  (a) barriers are
# switched to the sem-only (EventSemaphore, ge-wait) form, (b) the Tile exit
# drain's global-clock waits go onto a chain of single-wait NOPs, (c) a
# post-pass splits any remaining multi-wait instruction into single-wait NOPs
# on the same engine.

_PATCHED = False


def _apply_patches():
    global _PATCHED
    if _PATCHED:
        return
    _PATCHED = True

    def _sem_only_meb(self, engines):
        for inst in self._sem_only_all_engine_barrier_insts("aeb"):
            self.engines[inst.engine].add_instruction(inst)

    def _sem_only_aeb(self, *, sem_only=False):
        _sem_only_meb(self, None)

    bass.Bass.multi_engine_barrier = _sem_only_meb
    bass.Bass.all_engine_barrier = _sem_only_aeb

    try:
        from concourse.tile import ScopedClock
    except ImportError:
        from concourse.tile_sem_assignment import ScopedClock

    def _drain_and_barrier(self, tick_clock, wait_clock):
        nc = self.nc
        carrier = nc.sync.nop()
        wait_clock.add_sem_waits(
            carrier.ins, ScopedClock({None: tick_clock.global_clock})
        )
        # the split pass below breaks the carrier's waits into 1-wait nops
        nc.sync.drain()
        nc.all_engine_barrier()
        assert self.sems is not None
        popped = nc._tile_sem_poison_stack.pop()
        assert popped is self._sem_poison
        nc.clear_and_free_semaphores(list(self.sems.allocated().values()))
        nc.all_engine_barrier()

    tile.TileContext._drain_and_barrier = _drain_and_barrier


def _split_multiwaits(nc):
    """Hoist all-but-one sync wait of every instruction onto fresh NOPs
    placed immediately before it on the same engine."""
    ctr = 0
    for f in nc.m.functions:
        for bb in f.blocks:
            insts = bb.instructions
            out = []
            changed = False
            for ins in insts:
                si = ins.sync_info
                waits = list(si.on_wait) if si is not None and si.on_wait else []
                if len(waits) > 1:
                    changed = True
                    for w in waits[:-1]:
                        ctr += 1
                        nop = mybir.InstNoOp(name=f"wsplit-{ctr}", ins=[], outs=[])
                        nop.engine = ins.engine
                        nop.sync_info = mybir.SyncInfo(on_wait=[w], on_update=[])
                        nc.register_instruction(nop)
                        out.append(nop)
                    si.on_wait = [waits[-1]]
                out.append(ins)
            if changed:
                bb.instructions = out


# ---------------------------------------------------------------------------
# device program


def build_program(evict_dve_mod=4):
    """Build the SPMD Bass program (identical on all 8 cores)."""
    _apply_patches()
    nc = bass.Bass(trn_type="TRN2", name="c2f_critic")

    p_in = nc.dram_tensor(
        "p", [N_CHUNKS, 115, SLOTS, ROWS], F32, kind="ExternalInput"
    )
    rd_in = nc.dram_tensor("rd", [128, 2 * (B_CORE + SLOTS)], F32, kind="ExternalInput")
    # consts layout (free axis): 0 -> z2 (support on both partition bands),
    # 1..51 -> iota 0..50, 52 -> 1.0, 53..76 -> low0 tiled, 77..100 -> high0
    consts_in = nc.dram_tensor("consts", [128, 101], F32, kind="ExternalInput")
    act_in = nc.dram_tensor("act", [128, 4 * ACTION_DIM], F32, kind="ExternalInput")

    out_t = nc.dram_tensor(
        "out", [N_CHUNKS, 115, SLOTS, ROWS], F32, kind="ExternalOutput"
    )
    dec_t = nc.dram_tensor("dec", [128, 4 * ACTION_DIM], F32, kind="ExternalOutput")

    INV_DZ = float(1.0 / np.float64(DELTA_Z))  # 2.5 exactly

    with tile.TileContext(nc) as tc:
        with (
            tc.tile_pool(name="const", bufs=1) as const_pool,
            tc.tile_pool(name="bcomp", bufs=1) as b_pool,
            tc.tile_pool(name="inp", bufs=6) as in_pool,
            tc.tile_pool(name="wgen", bufs=2) as w_pool,
            tc.tile_pool(name="wtmp", bufs=2) as wtmp_pool,
            tc.tile_pool(name="outp", bufs=4) as out_pool,
            tc.tile_pool(name="dec", bufs=1) as dec_pool,
            tc.tile_pool(name="ps", bufs=1, space="PSUM") as psum_pool,
        ):
            AL = mybir.AluOpType

            cst = const_pool.tile([128, 101], F32)
            nc.sync.dma_start(cst[:, :], consts_in[:, :])
            one_b = cst[:, 52:53]

            # 4 persistent 2-bank PSUM tiles, manually rotated.  Row-band-0
            # matmuls write bank 0, row-band-64 matmuls write bank 1 —
            # concurrent PE writes to the same bank AND partition range hang
            # the device.  Zero once so evictions read defined data in the
            # junk partition band (51..63).
            ps_tiles = [
                psum_pool.tile([128, 1024], F32, tag=f"ps{i}", name=f"psb{i}")
                for i in range(4)
            ]
            for t in ps_tiles:
                nc.vector.memset(t[:, :], 0.0)

            # ---- b = clip((r + d*z_j - VMIN) * INV_DZ, 0, 50), both bands.
            # rd is DMA'd once to partition 0 and broadcast on-chip (Pool);
            # band-64 partitions use columns SHIFTED by SLOTS so one W-gen
            # instruction per chunk covers both bands.
            RDW = B_CORE + SLOTS
            rd_bc = b_pool.tile([128, 2 * RDW], F32, tag="rdbc")
            nc.sync.dma_start(rd_bc[0:115, :], rd_in[0:115, :])
            t1 = b_pool.tile([128, B_CORE], F32, tag="bt1")
            ball = b_pool.tile([128, B_CORE], F32, tag="ball")
            for pa, pz, sh in ((0, 64, 0), (64, 64 + ATOMS, SLOTS)):
                rbc = rd_bc[pa:pz, sh : sh + B_CORE]
                dbc = rd_bc[pa:pz, RDW + sh : RDW + sh + B_CORE]
                # t1 = d * z_j
                nc.vector.tensor_scalar(
                    t1[pa:pz, :], dbc, cst[pa:pz, 0:1], None, AL.mult
                )
                # t1 = t1 + r  (same order as reference's r + d*z)
                nc.vector.tensor_tensor(t1[pa:pz, :], t1[pa:pz, :], rbc, AL.add)
                # ball = (t1 - VMIN) * INV_DZ
                nc.vector.tensor_scalar(
                    ball[pa:pz, :], t1[pa:pz, :], V_MIN, INV_DZ, AL.subtract, AL.mult
                )
                # ball = clip(ball, 0, 50)
                nc.vector.tensor_scalar(
                    ball[pa:pz, :], ball[pa:pz, :], 0.0, float(ATOMS - 1),
                    AL.max, AL.min,
                )

            # ---- decoded (interval-refinement encode+decode, elementwise)
            AD4 = 4 * ACTION_DIM
            act = dec_pool.tile([128, AD4], F32, tag="act")
            nc.sync.dma_start(act[:, :], act_in[:, :])
            low = dec_pool.tile([128, AD4], F32, tag="low")
            high = dec_pool.tile([128, AD4], F32, tag="high")
            sr = dec_pool.tile([128, AD4], F32, tag="sr")
            q = dec_pool.tile([128, AD4], F32, tag="q")
            rsr = dec_pool.tile([128, AD4], F32, tag="rsr")
            cmp = dec_pool.tile([128, AD4 * (BINS - 1)], F32, tag="cmp")
            idx = dec_pool.tile([128, AD4], F32, tag="idx")
            nc.vector.tensor_copy(low[:, :], cst[:, 53 : 53 + AD4])
            nc.vector.tensor_copy(high[:, :], cst[:, 77 : 77 + AD4])
            INV_BINS = float(np.float32(1.0) / np.float32(BINS))
            for _lvl in range(LEVELS):
                # sr = (high - low) / BINS  (via exact-constant reciprocal)
                nc.vector.tensor_tensor(sr[:, :], high[:, :], low[:, :], AL.subtract)
                nc.vector.tensor_scalar(sr[:, :], sr[:, :], INV_BINS, None, AL.mult)
                # q = (act - low) * (1/sr)
                nc.vector.tensor_tensor(q[:, :], act[:, :], low[:, :], AL.subtract)
                nc.vector.reciprocal(rsr[:, :], sr[:, :])
                nc.vector.tensor_tensor(q[:, :], q[:, :], rsr[:, :], AL.mult)
                # idx = clip(floor(q), 0, 8) = sum_m [q >= m], m = 1..8 (exact)
                cv = cmp[:, :].rearrange("p (a m) -> p a m", m=BINS - 1)
                q_b = q[:, :].unsqueeze(2).broadcast_to((128, AD4, BINS - 1))
                thr_b = cst[:, 2:10].unsqueeze(1).broadcast_to((128, AD4, BINS - 1))
                nc.vector.tensor_tensor(cv, q_b, thr_b, AL.is_ge)
                nc.vector.tensor_reduce(
                    idx[:, :], cv, mybir.AxisListType.X, AL.add
                )
                # cont = low + sr*idx ; low = max(-1, cont); high = min(1, cont+sr)
                nc.vector.tensor_tensor(q[:, :], sr[:, :], idx[:, :], AL.mult)
                nc.vector.tensor_tensor(q[:, :], low[:, :], q[:, :], AL.add)
                nc.vector.tensor_scalar(low[:, :], q[:, :], -1.0, None, AL.max)
                nc.vector.tensor_tensor(high[:, :], q[:, :], sr[:, :], AL.add)
                nc.vector.tensor_scalar(high[:, :], high[:, :], 1.0, None, AL.min)
            dec = dec_pool.tile([128, AD4], F32, tag="decout")
            nc.vector.tensor_tensor(dec[:, :], high[:, :], low[:, :], AL.add)
            nc.vector.tensor_scalar(dec[:, :], dec[:, :], 0.5, None, AL.mult)
            nc.gpsimd.dma_start(dec_t[:, :], dec[:, :])

            # ---- main loop over chunks
            for c in range(N_CHUNKS):
                tin = in_pool.tile([128, SLOTS * ROWS], F32, tag="tin")
                HF = SLOTS * ROWS // 2
                nc.sync.dma_start(
                    tin[0:115, 0:HF],
                    p_in[c, :, 0 : SLOTS // 2].rearrange("a s r -> a (s r)"),
                )
                nc.sync.dma_start(
                    tin[0:115, HF:],
                    p_in[c, :, SLOTS // 2 :].rearrange("a s r -> a (s r)"),
                )

                # W generation, both bands in one pass (partitions 0..114;
                # the junk band 51..63 computes garbage that is never read)
                wt = w_pool.tile([128, SLOTS * ATOMS], F32, tag="wt")
                cols = ball[0:115, c * CHUNK : c * CHUNK + SLOTS]
                diff = wtmp_pool.tile([128, SLOTS * ATOMS], F32, tag="diff")
                dv = diff[0:115, :].rearrange("p (e t) -> p e t", t=ATOMS)
                iota_b = cst[0:115, 1 : 1 + ATOMS].unsqueeze(1).broadcast_to(
                    (115, SLOTS, ATOMS)
                )
                b_b = cols.unsqueeze(2).broadcast_to((115, SLOTS, ATOMS))
                nc.vector.tensor_tensor(dv, iota_b, b_b, AL.subtract)
                # y = |diff| = max(-diff, diff)
                nc.vector.scalar_tensor_tensor(
                    diff[0:115, :], diff[0:115, :], -1.0, diff[0:115, :],
                    AL.mult, AL.max,
                )
                # W = relu(1 - y)
                nc.scalar.activation(
                    wt[0:115, :], diff[0:115, :],
                    mybir.ActivationFunctionType.Relu,
                    bias=one_b[0:115, :], scale=-1.0,
                )
                w0 = wt
                w64 = wt

                tout = out_pool.tile([128, GROUPS * 2 * ROWS], F32, tag="tout")
                for k in range(GROUPS):
                    ps = ps_tiles[(c * GROUPS + k) % 4]
                    s0, s1 = 2 * k, 2 * k + 1
                    # four concurrent [51,51]x[51,162] matmuls, one per quadrant;
                    # row-band-0 pair in bank 0, row-band-64 pair in bank 1
                    nc.tensor.matmul(
                        ps[0:ATOMS, 0:ROWS],
                        w0[0:ATOMS, s0 * ATOMS : (s0 + 1) * ATOMS],
                        tin[0:ATOMS, s0 * ROWS : (s0 + 1) * ROWS],
                    )
                    nc.tensor.matmul(
                        ps[64 : 64 + ATOMS, 0:ROWS],
                        w0[0:ATOMS, s1 * ATOMS : (s1 + 1) * ATOMS],
                        tin[0:ATOMS, s1 * ROWS : (s1 + 1) * ROWS],
                    )
                    nc.tensor.matmul(
                        ps[0:ATOMS, 512 : 512 + ROWS],
                        w64[64 : 64 + ATOMS, s0 * ATOMS : (s0 + 1) * ATOMS],
                        tin[64 : 64 + ATOMS, s0 * ROWS : (s0 + 1) * ROWS],
                    )
                    nc.tensor.matmul(
                        ps[64 : 64 + ATOMS, 512 : 512 + ROWS],
                        w64[64 : 64 + ATOMS, s1 * ATOMS : (s1 + 1) * ATOMS],
                        tin[64 : 64 + ATOMS, s1 * ROWS : (s1 + 1) * ROWS],
                    )
                    dst = tout[0:115, k * 2 * ROWS : (k + 1) * 2 * ROWS].rearrange(
                        "p (b f) -> p b f", b=2
                    )
                    src = ps[0:115, :].rearrange("p (b f) -> p b f", b=2)[
                        :, :, 0:ROWS
                    ]
                    if k % evict_dve_mod == 0:
                        nc.vector.tensor_copy(dst, src)
                    else:
                        nc.scalar.copy(dst, src)

                nc.gpsimd.dma_start(
                    out_t[c].rearrange("a s r -> a (s r)"), tout[0:115, :]
                )

    _split_multiwaits(nc)
    return nc


# ---------------------------------------------------------------------------
# host-side data marshalling

def _build_consts(support, initial_low, initial_high):
    consts = np.zeros((128, 101), dtype=np.float32)
    z = np.asarray(support, dtype=np.float32).reshape(ATOMS)
    consts[0:ATOMS, 0] = z
    consts[64 : 64 + ATOMS, 0] = z
    iota = np.arange(ATOMS, dtype=np.float32)
    consts[:, 1 : 1 + ATOMS] = iota[None, :]
    consts[:, 52] = 1.0
    lo = np.tile(np.asarray(initial_low, np.float32).reshape(ACTION_DIM), 4)
    hi = np.tile(np.asarray(initial_high, np.float32).reshape(ACTION_DIM), 4)
    consts[:, 53:77] = lo[None, :]
    consts[:, 77:101] = hi[None, :]
    return consts


def _prep_core_inputs(p_core, r_core, d_core, a_core, consts):
    """p_core [512,162,51]; r/d [512]; a_core [512,6] -> in_map dict."""
    # dev elem order within a chunk: band0 = 4k+s, band64 = 4k+2+s (k<16,s<2);
    # partition rows 51..63 are padding (never read by the matmuls)
    x = p_core.reshape(N_CHUNKS, GROUPS, 2, 2, ROWS, ATOMS).transpose(0, 2, 5, 1, 3, 4)
    p_dev = np.empty((N_CHUNKS, 115, SLOTS, ROWS), np.float32)
    p_dev[:, 0:ATOMS] = x[:, 0].reshape(N_CHUNKS, ATOMS, SLOTS, ROWS)
    p_dev[:, 64:115] = x[:, 1].reshape(N_CHUNKS, ATOMS, SLOTS, ROWS)
    RDW = B_CORE + SLOTS
    rd1 = np.zeros((2, RDW), np.float32)
    r4 = r_core.reshape(N_CHUNKS, GROUPS, 2, 2).transpose(0, 2, 1, 3)
    d4 = d_core.reshape(N_CHUNKS, GROUPS, 2, 2).transpose(0, 2, 1, 3)
    rd1[0, 0:B_CORE] = r4.reshape(B_CORE)
    rd1[1, 0:B_CORE] = d4.reshape(B_CORE)
    rd1[1, B_CORE:] = 0.5  # pad: harmless nonzero discount
    rd = np.ascontiguousarray(
        np.broadcast_to(rd1.reshape(1, 2 * RDW), (128, 2 * RDW))
    )
    act = np.ascontiguousarray(a_core.reshape(128, 4 * ACTION_DIM))
    return {"p": p_dev, "rd": rd, "consts": consts, "act": act}


def _unpack_core_out(out_dev, dec_dev):
    """out_dev [8,2,51,32,162] -> [512,162,51]; dec [128,24] -> [512,6].
    outband0 slot (k,s2) = elem 4k+2*s2; outband64 = 4k+1+2*s2."""
    y = np.stack([out_dev[:, 0:ATOMS], out_dev[:, 64:115]], axis=1).reshape(
        N_CHUNKS, 2, ATOMS, GROUPS, 2, ROWS
    )
    proj = np.ascontiguousarray(y.transpose(0, 3, 4, 1, 5, 2)).reshape(
        B_CORE, ROWS, ATOMS
    )
    dec = dec_dev.reshape(B_CORE, ACTION_DIM)
    return proj, dec


_CACHED_NC = None


def _get_nc():
    global _CACHED_NC
    if _CACHED_NC is None:
        _CACHED_NC = build_program()
    return _CACHED_NC


def run(inputs, trace=False):
    """Returns ((projected, decoded), exec_time_ns_or_None)."""
    nqp = np.asarray(inputs["next_q_probs"], np.float32)
    reward = np.asarray(inputs["reward"], np.float32).reshape(B)
    discount = np.asarray(inputs["discount"], np.float32).reshape(B)
    caction = np.asarray(inputs["continuous_action"], np.float32)
    support = np.asarray(inputs["support"], np.float32)
    ilow = np.asarray(inputs["initial_low"], np.float32)
    ihigh = np.asarray(inputs["initial_high"], np.float32)

    consts = _build_consts(support, ilow, ihigh)
    p_all = nqp.reshape(N_CORES, B_CORE, ROWS, ATOMS)
    r_all = reward.reshape(N_CORES, B_CORE)
    d_all = discount.reshape(N_CORES, B_CORE)
    a_all = caction.reshape(N_CORES, B_CORE, ACTION_DIM)

    in_maps = [
        _prep_core_inputs(p_all[c], r_all[c], d_all[c], a_all[c], consts)
        for c in range(N_CORES)
    ]

    nc = _get_nc()
    res = run_bass_kernel_spmd(
        nc, in_maps, core_ids=list(range(N_CORES)), trace=trace
    )

    proj = np.empty((B, ROWS, ATOMS), np.float32)
    dec = np.empty((B, ACTION_DIM), np.float32)
    for c in range(N_CORES):
        pc, dc = _unpack_core_out(res.results[c]["out"], res.results[c]["dec"])
        proj[c * B_CORE : (c + 1) * B_CORE] = pc
        dec[c * B_CORE : (c + 1) * B_CORE] = dc

    projected = proj.reshape(B, LEVELS, ACTION_DIM, BINS, ATOMS)
    return (projected, dec), res.exec_time_ns


def kernel(**inputs):
    (projected, decoded), _ = run(inputs, trace=bool(os.environ.get("BASS_KERNEL_TRACE")))
    return projected, decoded


# revision 28
# speedup vs baseline: 1.0608x; 1.0044x over previous
"""Trainium2 Bass kernel for the C2F critic head (C51 Bellman projection +
interval-refinement action decode).

Math: the reference's per-row scatter-add projection is a per-batch-element
linear map: projected[b] = p_rows[b] @ W_b, with the hat-function matrix
W_b[j, t] = relu(1 - |b_j - t|), b_j = clip((r_b + d_b*z_j - VMIN)/dz, 0, 50).
The fixup-laden lower/upper scatter in the reference is exactly this hat
function (verified to ~1e-6 rel against the jax reference).

Device mapping (per core, 512 batch elems, 16 chunks of 32):
  - host pre-permutes p to [chunk][115 partition rows][16 slots][162 rows]
    (atom bands at partition rows 0..50 and 64..114; rows 51..63 padding)
    so each chunk is ONE contiguous ~1.2MB DMA touching 115 partitions —
    DMA bandwidth scales with partitions touched per transfer.
  - 4 batch elems run concurrently on the PE array as [51,51] x [51,162]
    matmuls placed in the four 64x64 quadrants (tile_position bases {0,64}).
  - W is generated on-chip: DVE broadcast-subtract + (abs_max, sub 1),
    ScalarE Relu(-x); b is computed once for all 512 elems.
  - PSUM eviction via DVE/ScalarE copies (DMA cannot read PSUM).
"""

import os
import sys

sys.path.insert(0, "/opt/trn_rl_repo")

import numpy as np

import concourse.bass as bass
import concourse.tile as tile
from concourse import mybir
from concourse.bass_utils import run_bass_kernel_spmd

# ---------------------------------------------------------------------------
# problem constants (from the reference module; fixed for this problem)
B = 4096
LEVELS = 3
BINS = 9
ATOMS = 51
ACTION_DIM = 6
ROWS = LEVELS * ACTION_DIM * BINS  # 162
V_MIN = -10.0
V_MAX = 10.0
DELTA_Z = (V_MAX - V_MIN) / (ATOMS - 1)

N_CORES = 8
B_CORE = B // N_CORES            # 512
CHUNK = 32                       # batch elems per chunk
N_CHUNKS = B_CORE // CHUNK       # 8
GROUPS = CHUNK // 4              # 16 4-elem matmul groups per chunk
SLOTS = CHUNK // 2               # 32 elems per band per chunk

F32 = mybir.dt.float32

# ---------------------------------------------------------------------------
# Toolchain workarounds: this walrus build accepts at most ONE ge-mode sync
# wait per instruction and rejects eq-mode waits entirely.  (a) barriers are
# switched to the sem-only (EventSemaphore, ge-wait) form, (b) the Tile exit
# drain's global-clock waits go onto a chain of single-wait NOPs, (c) a
# post-pass splits any remaining multi-wait instruction into single-wait NOPs
# on the same engine.

_PATCHED = False


def _apply_patches():
    global _PATCHED
    if _PATCHED:
        return
    _PATCHED = True

    def _sem_only_meb(self, engines):
        for inst in self._sem_only_all_engine_barrier_insts("aeb"):
            self.engines[inst.engine].add_instruction(inst)

    def _sem_only_aeb(self, *, sem_only=False):
        _sem_only_meb(self, None)

    bass.Bass.multi_engine_barrier = _sem_only_meb
    bass.Bass.all_engine_barrier = _sem_only_aeb

    try:
        from concourse.tile import ScopedClock
    except ImportError:
        from concourse.tile_sem_assignment import ScopedClock

    def _drain_and_barrier(self, tick_clock, wait_clock):
        nc = self.nc
        carrier = nc.sync.nop()
        wait_clock.add_sem_waits(
            carrier.ins, ScopedClock({None: tick_clock.global_clock})
        )
        # the split pass below breaks the carrier's waits into 1-wait nops
        nc.sync.drain()
        nc.all_engine_barrier()
        assert self.sems is not None
        popped = nc._tile_sem_poison_stack.pop()
        assert popped is self._sem_poison
        nc.clear_and_free_semaphores(list(self.sems.allocated().values()))
        nc.all_engine_barrier()

    tile.TileContext._drain_and_barrier = _drain_and_barrier


def _split_multiwaits(nc):
    """Hoist all-but-one sync wait of every instruction onto fresh NOPs
    placed immediately before it on the same engine."""
    ctr = 0
    for f in nc.m.functions:
        for bb in f.blocks:
            insts = bb.instructions
            out = []
            changed = False
            for ins in insts:
                si = ins.sync_info
                waits = list(si.on_wait) if si is not None and si.on_wait else []
                if len(waits) > 1:
                    changed = True
                    for w in waits[:-1]:
                        ctr += 1
                        nop = mybir.InstNoOp(name=f"wsplit-{ctr}", ins=[], outs=[])
                        nop.engine = ins.engine
                        nop.sync_info = mybir.SyncInfo(on_wait=[w], on_update=[])
                        nc.register_instruction(nop)
                        out.append(nop)
                    si.on_wait = [waits[-1]]
                out.append(ins)
            if changed:
                bb.instructions = out


# ---------------------------------------------------------------------------
# device program


def build_program(evict_dve_mod=4):
    """Build the SPMD Bass program (identical on all 8 cores)."""
    _apply_patches()
    nc = bass.Bass(trn_type="TRN2", name="c2f_critic")

    p_in = nc.dram_tensor(
        "p", [N_CHUNKS, 115, SLOTS, ROWS], F32, kind="ExternalInput"
    )
    rd_in = nc.dram_tensor("rd", [128, 2 * (B_CORE + SLOTS)], F32, kind="ExternalInput")
    # consts layout (free axis): 0 -> z2 (support on both partition bands),
    # 1..51 -> iota 0..50, 52 -> 1.0, 53..76 -> low0 tiled, 77..100 -> high0
    consts_in = nc.dram_tensor("consts", [128, 101], F32, kind="ExternalInput")
    act_in = nc.dram_tensor("act", [128, 4 * ACTION_DIM], F32, kind="ExternalInput")

    out_t = nc.dram_tensor(
        "out", [N_CHUNKS, 115, SLOTS, ROWS], F32, kind="ExternalOutput"
    )
    dec_t = nc.dram_tensor("dec", [128, 4 * ACTION_DIM], F32, kind="ExternalOutput")

    INV_DZ = float(1.0 / np.float64(DELTA_Z))  # 2.5 exactly

    with tile.TileContext(nc) as tc:
        with (
            tc.tile_pool(name="const", bufs=1) as const_pool,
            tc.tile_pool(name="bcomp", bufs=1) as b_pool,
            tc.tile_pool(name="inp", bufs=6) as in_pool,
            tc.tile_pool(name="wgen", bufs=2) as w_pool,
            tc.tile_pool(name="wtmp", bufs=2) as wtmp_pool,
            tc.tile_pool(name="outp", bufs=4) as out_pool,
            tc.tile_pool(name="dec", bufs=1) as dec_pool,
            tc.tile_pool(name="ps", bufs=1, space="PSUM") as psum_pool,
        ):
            AL = mybir.AluOpType

            cst = const_pool.tile([128, 101], F32)
            nc.sync.dma_start(cst[:, :], consts_in[:, :])
            one_b = cst[:, 52:53]

            # 4 persistent 2-bank PSUM tiles, manually rotated.  Row-band-0
            # matmuls write bank 0, row-band-64 matmuls write bank 1 —
            # concurrent PE writes to the same bank AND partition range hang
            # the device.  Zero once so evictions read defined data in the
            # junk partition band (51..63).
            ps_tiles = [
                psum_pool.tile([128, 1024], F32, tag=f"ps{i}", name=f"psb{i}")
                for i in range(4)
            ]
            for t in ps_tiles:
                nc.vector.memset(t[:, :], 0.0)

            # ---- b = clip((r + d*z_j - VMIN) * INV_DZ, 0, 50), both bands.
            # rd is DMA'd once to partition 0 and broadcast on-chip (Pool);
            # band-64 partitions use columns SHIFTED by SLOTS so one W-gen
            # instruction per chunk covers both bands.
            RDW = B_CORE + SLOTS
            rd_bc = b_pool.tile([128, 2 * RDW], F32, tag="rdbc")
            nc.sync.dma_start(rd_bc[0:115, :], rd_in[0:115, :])
            t1 = b_pool.tile([128, B_CORE], F32, tag="bt1")
            ball = b_pool.tile([128, B_CORE], F32, tag="ball")
            for pa, pz, sh in ((0, 64, 0), (64, 64 + ATOMS, SLOTS)):
                rbc = rd_bc[pa:pz, sh : sh + B_CORE]
                dbc = rd_bc[pa:pz, RDW + sh : RDW + sh + B_CORE]
                # t1 = d * z_j
                nc.vector.tensor_scalar(
                    t1[pa:pz, :], dbc, cst[pa:pz, 0:1], None, AL.mult
                )
                # t1 = t1 + r  (same order as reference's r + d*z)
                nc.vector.tensor_tensor(t1[pa:pz, :], t1[pa:pz, :], rbc, AL.add)
                # ball = (t1 - VMIN) * INV_DZ
                nc.vector.tensor_scalar(
                    ball[pa:pz, :], t1[pa:pz, :], V_MIN, INV_DZ, AL.subtract, AL.mult
                )
                # ball = clip(ball, 0, 50)
                nc.vector.tensor_scalar(
                    ball[pa:pz, :], ball[pa:pz, :], 0.0, float(ATOMS - 1),
                    AL.max, AL.min,
                )

            # ---- decoded (interval-refinement encode+decode, elementwise)
            AD4 = 4 * ACTION_DIM
            act = dec_pool.tile([128, AD4], F32, tag="act")
            nc.sync.dma_start(act[:, :], act_in[:, :])
            low = dec_pool.tile([128, AD4], F32, tag="low")
            high = dec_pool.tile([128, AD4], F32, tag="high")
            sr = dec_pool.tile([128, AD4], F32, tag="sr")
            q = dec_pool.tile([128, AD4], F32, tag="q")
            rsr = dec_pool.tile([128, AD4], F32, tag="rsr")
            cmp = dec_pool.tile([128, AD4 * (BINS - 1)], F32, tag="cmp")
            idx = dec_pool.tile([128, AD4], F32, tag="idx")
            nc.vector.tensor_copy(low[:, :], cst[:, 53 : 53 + AD4])
            nc.vector.tensor_copy(high[:, :], cst[:, 77 : 77 + AD4])
            INV_BINS = float(np.float32(1.0) / np.float32(BINS))
            for _lvl in range(LEVELS):
                # sr = (high - low) / BINS  (via exact-constant reciprocal)
                nc.vector.tensor_tensor(sr[:, :], high[:, :], low[:, :], AL.subtract)
                nc.vector.tensor_scalar(sr[:, :], sr[:, :], INV_BINS, None, AL.mult)
                # q = (act - low) * (1/sr)
                nc.vector.tensor_tensor(q[:, :], act[:, :], low[:, :], AL.subtract)
                nc.vector.reciprocal(rsr[:, :], sr[:, :])
                nc.vector.tensor_tensor(q[:, :], q[:, :], rsr[:, :], AL.mult)
                # idx = clip(floor(q), 0, 8) = sum_m [q >= m], m = 1..8 (exact)
                cv = cmp[:, :].rearrange("p (a m) -> p a m", m=BINS - 1)
                q_b = q[:, :].unsqueeze(2).broadcast_to((128, AD4, BINS - 1))
                thr_b = cst[:, 2:10].unsqueeze(1).broadcast_to((128, AD4, BINS - 1))
                nc.vector.tensor_tensor(cv, q_b, thr_b, AL.is_ge)
                nc.vector.tensor_reduce(
                    idx[:, :], cv, mybir.AxisListType.X, AL.add
                )
                # cont = low + sr*idx ; low = max(-1, cont); high = min(1, cont+sr)
                nc.vector.tensor_tensor(q[:, :], sr[:, :], idx[:, :], AL.mult)
                nc.vector.tensor_tensor(q[:, :], low[:, :], q[:, :], AL.add)
                nc.vector.tensor_scalar(low[:, :], q[:, :], -1.0, None, AL.max)
                nc.vector.tensor_tensor(high[:, :], q[:, :], sr[:, :], AL.add)
                nc.vector.tensor_scalar(high[:, :], high[:, :], 1.0, None, AL.min)
            dec = dec_pool.tile([128, AD4], F32, tag="decout")
            nc.vector.tensor_tensor(dec[:, :], high[:, :], low[:, :], AL.add)
            nc.vector.tensor_scalar(dec[:, :], dec[:, :], 0.5, None, AL.mult)
            nc.gpsimd.dma_start(dec_t[:, :], dec[:, :])

            # ---- main loop over chunks
            for c in range(N_CHUNKS):
                tin = in_pool.tile([128, SLOTS * ROWS], F32, tag="tin")
                HF = SLOTS * ROWS // 2
                nc.sync.dma_start(
                    tin[0:115, 0:HF],
                    p_in[c, :, 0 : SLOTS // 2].rearrange("a s r -> a (s r)"),
                )
                nc.sync.dma_start(
                    tin[0:115, HF:],
                    p_in[c, :, SLOTS // 2 :].rearrange("a s r -> a (s r)"),
                )

                # W generation, both bands in one pass (partitions 0..114;
                # the junk band 51..63 computes garbage that is never read)
                wt = w_pool.tile([128, SLOTS * ATOMS], F32, tag="wt")
                cols = ball[0:115, c * CHUNK : c * CHUNK + SLOTS]
                diff = wtmp_pool.tile([128, SLOTS * ATOMS], F32, tag="diff")
                dv = diff[0:115, :].rearrange("p (e t) -> p e t", t=ATOMS)
                iota_b = cst[0:115, 1 : 1 + ATOMS].unsqueeze(1).broadcast_to(
                    (115, SLOTS, ATOMS)
                )
                b_b = cols.unsqueeze(2).broadcast_to((115, SLOTS, ATOMS))
                nc.vector.tensor_tensor(dv, iota_b, b_b, AL.subtract)
                # y = |diff| = max(-diff, diff)
                nc.vector.scalar_tensor_tensor(
                    diff[0:115, :], diff[0:115, :], -1.0, diff[0:115, :],
                    AL.mult, AL.max,
                )
                # W = relu(1 - y)
                nc.scalar.activation(
                    wt[0:115, :], diff[0:115, :],
                    mybir.ActivationFunctionType.Relu,
                    bias=one_b[0:115, :], scale=-1.0,
                )
                w0 = wt
                w64 = wt

                tout = out_pool.tile([128, GROUPS * 2 * ROWS], F32, tag="tout")
                for k in range(GROUPS):
                    ps = ps_tiles[(c * GROUPS + k) % 4]
                    s0, s1 = 2 * k, 2 * k + 1
                    # four concurrent [51,51]x[51,162] matmuls, one per quadrant;
                    # row-band-0 pair in bank 0, row-band-64 pair in bank 1
                    nc.tensor.matmul(
                        ps[0:ATOMS, 0:ROWS],
                        w0[0:ATOMS, s0 * ATOMS : (s0 + 1) * ATOMS],
                        tin[0:ATOMS, s0 * ROWS : (s0 + 1) * ROWS],
                    )
                    nc.tensor.matmul(
                        ps[64 : 64 + ATOMS, 0:ROWS],
                        w0[0:ATOMS, s1 * ATOMS : (s1 + 1) * ATOMS],
                        tin[0:ATOMS, s1 * ROWS : (s1 + 1) * ROWS],
                    )
                    nc.tensor.matmul(
                        ps[0:ATOMS, 512 : 512 + ROWS],
                        w64[64 : 64 + ATOMS, s0 * ATOMS : (s0 + 1) * ATOMS],
                        tin[64 : 64 + ATOMS, s0 * ROWS : (s0 + 1) * ROWS],
                    )
                    nc.tensor.matmul(
                        ps[64 : 64 + ATOMS, 512 : 512 + ROWS],
                        w64[64 : 64 + ATOMS, s1 * ATOMS : (s1 + 1) * ATOMS],
                        tin[64 : 64 + ATOMS, s1 * ROWS : (s1 + 1) * ROWS],
                    )
                    dst = tout[0:115, k * 2 * ROWS : (k + 1) * 2 * ROWS].rearrange(
                        "p (b f) -> p b f", b=2
                    )
                    src = ps[0:115, :].rearrange("p (b f) -> p b f", b=2)[
                        :, :, 0:ROWS
                    ]
                    if k % evict_dve_mod == 0:
                        nc.vector.tensor_copy(dst, src)
                    else:
                        nc.scalar.copy(dst, src)

                nc.gpsimd.dma_start(
                    out_t[c].rearrange("a s r -> a (s r)"), tout[0:115, :]
                )

    _split_multiwaits(nc)
    return nc


# ---------------------------------------------------------------------------
# host-side data marshalling

def _build_consts(support, initial_low, initial_high):
    consts = np.zeros((128, 101), dtype=np.float32)
    z = np.asarray(support, dtype=np.float32).reshape(ATOMS)
    consts[0:ATOMS, 0] = z
    consts[64 : 64 + ATOMS, 0] = z
    iota = np.arange(ATOMS, dtype=np.float32)
    consts[:, 1 : 1 + ATOMS] = iota[None, :]
    consts[:, 52] = 1.0
    lo = np.tile(np.asarray(initial_low, np.float32).reshape(ACTION_DIM), 4)
    hi = np.tile(np.asarray(initial_high, np.float32).reshape(ACTION_DIM), 4)
    consts[:, 53:77] = lo[None, :]
    consts[:, 77:101] = hi[None, :]
    return consts


def _prep_core_inputs(p_core, r_core, d_core, a_core, consts):
    """p_core [512,162,51]; r/d [512]; a_core [512,6] -> in_map dict."""
    # dev elem order within a chunk: band0 = 4k+s, band64 = 4k+2+s (k<16,s<2);
    # partition rows 51..63 are padding (never read by the matmuls)
    x = p_core.reshape(N_CHUNKS, GROUPS, 2, 2, ROWS, ATOMS).transpose(0, 2, 5, 1, 3, 4)
    p_dev = np.empty((N_CHUNKS, 115, SLOTS, ROWS), np.float32)
    p_dev[:, 0:ATOMS] = x[:, 0].reshape(N_CHUNKS, ATOMS, SLOTS, ROWS)
    p_dev[:, 64:115] = x[:, 1].reshape(N_CHUNKS, ATOMS, SLOTS, ROWS)
    RDW = B_CORE + SLOTS
    rd1 = np.zeros((2, RDW), np.float32)
    r4 = r_core.reshape(N_CHUNKS, GROUPS, 2, 2).transpose(0, 2, 1, 3)
    d4 = d_core.reshape(N_CHUNKS, GROUPS, 2, 2).transpose(0, 2, 1, 3)
    rd1[0, 0:B_CORE] = r4.reshape(B_CORE)
    rd1[1, 0:B_CORE] = d4.reshape(B_CORE)
    rd1[1, B_CORE:] = 0.5  # pad: harmless nonzero discount
    rd = np.ascontiguousarray(
        np.broadcast_to(rd1.reshape(1, 2 * RDW), (128, 2 * RDW))
    )
    act = np.ascontiguousarray(a_core.reshape(128, 4 * ACTION_DIM))
    return {"p": p_dev, "rd": rd, "consts": consts, "act": act}


def _unpack_core_out(out_dev, dec_dev):
    """out_dev [8,2,51,32,162] -> [512,162,51]; dec [128,24] -> [512,6].
    outband0 slot (k,s2) = elem 4k+2*s2; outband64 = 4k+1+2*s2."""
    y = np.stack([out_dev[:, 0:ATOMS], out_dev[:, 64:115]], axis=1).reshape(
        N_CHUNKS, 2, ATOMS, GROUPS, 2, ROWS
    )
    proj = np.ascontiguousarray(y.transpose(0, 3, 4, 1, 5, 2)).reshape(
        B_CORE, ROWS, ATOMS
    )
    dec = dec_dev.reshape(B_CORE, ACTION_DIM)
    return proj, dec


_CACHED_NC = None


def _get_nc():
    global _CACHED_NC
    if _CACHED_NC is None:
        _CACHED_NC = build_program()
    return _CACHED_NC


def run(inputs, trace=False):
    """Returns ((projected, decoded), exec_time_ns_or_None)."""
    nqp = np.asarray(inputs["next_q_probs"], np.float32)
    reward = np.asarray(inputs["reward"], np.float32).reshape(B)
    discount = np.asarray(inputs["discount"], np.float32).reshape(B)
    caction = np.asarray(inputs["continuous_action"], np.float32)
    support = np.asarray(inputs["support"], np.float32)
    ilow = np.asarray(inputs["initial_low"], np.float32)
    ihigh = np.asarray(inputs["initial_high"], np.float32)

    consts = _build_consts(support, ilow, ihigh)
    p_all = nqp.reshape(N_CORES, B_CORE, ROWS, ATOMS)
    r_all = reward.reshape(N_CORES, B_CORE)
    d_all = discount.reshape(N_CORES, B_CORE)
    a_all = caction.reshape(N_CORES, B_CORE, ACTION_DIM)

    in_maps = [
        _prep_core_inputs(p_all[c], r_all[c], d_all[c], a_all[c], consts)
        for c in range(N_CORES)
    ]

    nc = _get_nc()
    res = run_bass_kernel_spmd(
        nc, in_maps, core_ids=list(range(N_CORES)), trace=trace
    )

    proj = np.empty((B, ROWS, ATOMS), np.float32)
    dec = np.empty((B, ACTION_DIM), np.float32)
    for c in range(N_CORES):
        pc, dc = _unpack_core_out(res.results[c]["out"], res.results[c]["dec"])
        proj[c * B_CORE : (c + 1) * B_CORE] = pc
        dec[c * B_CORE : (c + 1) * B_CORE] = dc

    projected = proj.reshape(B, LEVELS, ACTION_DIM, BINS, ATOMS)
    return (projected, dec), res.exec_time_ns


def kernel(**inputs):
    (projected, decoded), _ = run(inputs, trace=bool(os.environ.get("BASS_KERNEL_TRACE")))
    return projected, decoded
